# revision 1
# baseline (speedup 1.0000x reference)
"""Trainium2 Bass kernel for nn_Code_Multiplexing — v6.

Math: per batch, a fixed 32x32 +/-1 map A over the 32 floats (4 streams x
4 l x re/im); y = A x. Per-core batch shard of 65536.

v6 = v5 (fp16 matmul path with batch-on-PSUM-partitions + cheap stores)
plus a Pool-engine butterfly side-channel: the last 8192 batches per core
are computed directly in SBUF with 10 fp16 tensor_tensor ops on the Pool
engine (2-stage radix-2 butterfly), bypassing PSUM entirely. That sheds
2048 columns from the PSUM->SBUF copy chain (DVE+ACT), which is the
binding resource, and uses Pool queue slack.
"""

import numpy as np

P = 128
B_FULL = 524288
N_CORES = 8
B_CORE = B_FULL // N_CORES      # 65536
FEAT = 32
M_B = 64                        # butterfly batches per partition
NB = P * M_B                    # 8192 butterfly batches per core
B_MAIN = B_CORE - NB            # 57344 matmul-path batches
COLS = B_MAIN // 4              # 14336 matmul columns (4 batches each)
OPAD = 130                      # padded row length of the main output
OPPAD = 33                      # padded row length of the butterfly output

_CACHE = {}


def _amatrix():
    Z = np.array(
        [[1, 1, 1, 1], [1j, -1j, 1j, -1j], [1, 1, -1, -1], [1j, -1j, -1j, 1j]],
        dtype=np.complex64,
    )
    A = np.zeros((FEAT, FEAT), np.float32)
    for o in range(4):
        for k in range(4):
            for j in range(4):
                re, im = float(Z[k, j].real), float(Z[k, j].imag)
                A[o * 8 + k * 2 + 0, j * 8 + o * 2 + 0] = re
                A[o * 8 + k * 2 + 0, j * 8 + o * 2 + 1] = -im
                A[o * 8 + k * 2 + 1, j * 8 + o * 2 + 0] = im
                A[o * 8 + k * 2 + 1, j * 8 + o * 2 + 1] = re
    return A


def _weight_matrix():
    # W[k = b_lo*32+f_in, n = b_lo*32+f_out] = A[f_out, f_in]
    A = _amatrix()
    W = np.zeros((P, P), np.float16)
    for blo in range(4):
        W[blo * 32:(blo + 1) * 32, blo * 32:(blo + 1) * 32] = A.T.astype(np.float16)
    return W


def _build_nc():
    import concourse.bacc as bacc
    import concourse.mybir as mybir
    from concourse.tile import TileContext

    f32 = mybir.dt.float32
    fp16 = mybir.dt.float16
    add = mybir.AluOpType.add
    sub = mybir.AluOpType.subtract
    nc = bacc.Bacc(None, target_bir_lowering=False)

    x = nc.dram_tensor("x", [P, COLS], fp16, kind="ExternalInput")
    xp = nc.dram_tensor("xp", [P, FEAT * M_B], fp16, kind="ExternalInput")
    w = nc.dram_tensor("w", [P, P], fp16, kind="ExternalInput")
    out = nc.dram_tensor("out", [COLS, OPAD], fp16, kind="ExternalOutput")
    outp = nc.dram_tensor("outp", [NB, OPPAD], fp16, kind="ExternalOutput")

    with TileContext(nc) as tc:
        with (
            tc.tile_pool(name="wpool", bufs=1) as wpool,
            tc.tile_pool(name="pool", bufs=1) as pool,
            tc.tile_pool(name="psum", bufs=4, space="PSUM") as psum_pool,
        ):
            X = pool.tile([P, COLS], fp16, name="x_t")
            XP = pool.tile([P, FEAT * M_B], fp16, name="xp_t")
            T1 = pool.tile([P, FEAT * M_B], fp16, name="t1")
            GP = pool.tile([P, FEAT * M_B], fp16, name="gp")
            G0 = pool.tile([P, COLS // 2], fp16, name="g0")
            G1 = pool.tile([P, COLS // 2], fp16, name="g1")
            Wt = wpool.tile([P, P], fp16, name="w_t")
            # first x pieces on SP/Pool; W rides the otherwise-idle ACT
            # HWDGE queue so it lands in parallel with them
            nc.sync.dma_start(out=X[:, :512], in_=x[:, :512])
            nc.scalar.dma_start(out=Wt[:], in_=w[:])
            # warm-up: start the PE p-state ramp clock and pull the ACT
            # activation-table load off the critical path, both during fill
            Wm = wpool.tile([P, 2], fp16, name="w_warm")
            nc.vector.memset(Wm[:], 0.0)
            Wm2 = wpool.tile([P, 2], fp16, name="w_warm2")
            nc.scalar.copy(Wm2[:], Wm[:])
            ps0 = psum_pool.tile([P, 2], f32, tag="ps", name="ps_warm",
                                 padded_shape=[P, 1024])
            nc.tensor.matmul(ps0[:2, :2], Wm[:, :2], Wm[:, :2],
                             start=True, stop=True)

            # remaining main loads; Pool also runs the butterfly + 2 stores,
            # so SP carries more pieces
            sp_pieces = [512, 512] + [1024] * 9
            pl_pieces = [512] + [1024] * 3
            assert sum(sp_pieces) + sum(pl_pieces) == COLS - 512
            col = 512
            pieces = []
            for i in range(max(len(sp_pieces), len(pl_pieces))):
                if i < len(sp_pieces):
                    pieces.append((nc.sync, sp_pieces[i]))
                if i < len(pl_pieces):
                    pieces.append((nc.gpsimd, pl_pieces[i]))
            npool = 0
            for eng, ln in pieces:
                eng.dma_start(out=X[:, col:col + ln], in_=x[:, col:col + ln])
                col += ln
                if eng is nc.gpsimd:
                    npool += 1
                    if npool == 1:
                        # butterfly input rides Pool after its first x piece
                        nc.gpsimd.dma_start(out=XP[:], in_=xp[:])
            assert col == COLS

            # ---- Pool butterfly over the last NB batches ----
            # XP feature blocks (4m wide each, layout (l, q) q-fastest):
            #   [a0, a1, a2, a3, b0, b1, b2, b3]  (a=re, b=im, index=stream j)
            # T1 blocks: [u1, u3, w1, v1, u2, u4, w2, v2]
            m4 = 4 * M_B

            def blk(t, i):
                return t[:, i * m4:(i + 1) * m4]

            s1 = [(0, 1, add, 0), (4, 5, add, 1), (5, 4, sub, 2), (0, 1, sub, 3),
                  (2, 3, add, 4), (6, 7, add, 5), (7, 6, sub, 6), (2, 3, sub, 7)]
            for i1, i2, op, o in s1:
                nc.gpsimd.tensor_tensor(blk(T1, o), blk(XP, i1), blk(XP, i2), op=op)
            # stage 2: [u1,u3,w1,v1] +/- [u2,u4,w2,v2] ->
            #   add: [reY0, imY0, reY1, imY1] -> f_out = l*8 + (0..3)
            #   sub: [reY2, imY2, reY3, imY3] -> f_out = l*8 + (4..7)
            in1 = T1[:, :4 * m4]
            in2 = T1[:, 4 * m4:]
            gp4 = GP[:].rearrange("p (q l c) -> p c l q", l=4, c=8)
            nc.gpsimd.tensor_tensor(gp4[:, 0:4], in1, in2, op=add)
            nc.gpsimd.tensor_tensor(gp4[:, 4:8], in1, in2, op=sub)
            # two halves: 6144 descriptors each stays under the loader's
            # static-ring limit
            nc.gpsimd.dma_start(out=outp[:NB // 2, :FEAT], in_=GP[:, :FEAT * M_B // 2])
            nc.gpsimd.dma_start(out=outp[NB // 2:, :FEAT], in_=GP[:, FEAT * M_B // 2:])

            # ---- matmul path: copy units (PSUM depth 4 at 1024) ----
            units = [512] * 4 + [1024] * 5 + [1024] * 5 + [512] * 4
            assert sum(units) == COLS
            busy = {"dve": 0.0, "act": -400.0}
            ucol = 0
            for ui, ln in enumerate(units):
                ps = psum_pool.tile([P, ln], f32, tag="ps", name="ps",
                                    padded_shape=[P, 1024])
                for i in range(ln // 128):
                    c = ucol // 128 + i
                    nc.tensor.matmul(ps[:, i * 128:(i + 1) * 128],
                                     X[:, c * 128:(c + 1) * 128], Wt[:],
                                     start=(i % 4 == 0), stop=(i % 4 == 3),
                                     skip_group_check=True)
                Gh, off = (G0, ucol) if ucol < COLS // 2 else (G1, ucol - COLS // 2)
                if ui == len(units) - 1:
                    # final unit: split across both engines for a short drain
                    h = ln // 2
                    nc.vector.tensor_copy(Gh[:, off:off + h], ps[:, :h])
                    nc.scalar.copy(Gh[:, off + h:off + ln], ps[:, h:])
                else:
                    cost_d = ln * 1.042 + 125
                    cost_a = ln * 0.833 + 185
                    if busy["dve"] + cost_d <= busy["act"] + cost_a:
                        busy["dve"] += cost_d
                        nc.vector.tensor_copy(Gh[:, off:off + ln], ps[:])
                    else:
                        busy["act"] += cost_a
                        nc.scalar.copy(Gh[:, off:off + ln], ps[:])
                ucol += ln

            # stores: plain [128, 7680] SBUF -> row-strided DRAM (130-elem
            # rows, 128 used); out AP free bytes 256 -> 500ns floor.
            # G0 completes mid-run (Pool queue fine); G1 is the tail store,
            # so it rides SP whose HWDGE init latency is ~170ns lower.
            nc.gpsimd.dma_start(out=out[:COLS // 2, :128], in_=G0[:])
            nc.sync.dma_start(out=out[COLS // 2:, :128], in_=G1[:])
    nc.compile()
    return nc


def _get_nc():
    if "nc" not in _CACHE:
        _CACHE["nc"] = _build_nc()
    return _CACHE["nc"]


def kernel(x0, x1, x2, x3):
    from concourse.bass_utils import run_bass_kernel_spmd

    xs = [np.asarray(a, dtype=np.float32) for a in (x0, x1, x2, x3)]
    arr = np.stack(xs)                                  # [4j, B, 4l, 2r]
    W = _weight_matrix()
    nc = _get_nc()
    in_maps = []
    for c in range(N_CORES):
        sl = arr[:, c * B_CORE:(c + 1) * B_CORE]        # [4j, B_CORE, 4, 2]
        # feature f = j*8 + l*2 + r ; main batch b -> (col=b//4, b_lo=b%4)
        v = sl.transpose(1, 0, 2, 3).reshape(B_CORE, FEAT).astype(np.float16)
        vm = v[:B_MAIN]
        xdev = (vm.reshape(COLS, 4, FEAT)               # [col, b_lo, f]
                 .transpose(1, 2, 0)                    # [b_lo, f, col]
                 .reshape(P, COLS))
        # butterfly batches: b_P -> (p = b_P%128, q = b_P//128)
        # XP[p, (r*4+j)*4m + l*m + q]   (f_in = j*8 + l*2 + r)
        vp = v[B_MAIN:].reshape(M_B, P, 4, 4, 2)        # [q, p, j, l, r]
        xpd = (vp.transpose(1, 4, 2, 3, 0)              # [p, r, j, l, q]
                 .reshape(P, FEAT * M_B))
        in_maps.append({"x": np.ascontiguousarray(xdev),
                        "xp": np.ascontiguousarray(xpd), "w": W})
    res = run_bass_kernel_spmd(nc, in_maps, core_ids=list(range(N_CORES))).results
    parts = []
    half = COLS // 2
    for c in range(N_CORES):
        od = res[c]["out"][:, :128]                     # [row, (b_lo, f_out)]
        colmat = np.empty((COLS, P), od.dtype)
        for h in range(2):
            H = od[h * half:(h + 1) * half]
            H = H.reshape(P, half // P, P).transpose(1, 0, 2)   # [a, p, b]
            colmat[h * half:(h + 1) * half] = H.reshape(half, P)
        obm = colmat.reshape(COLS, 4, FEAT).reshape(B_MAIN, FEAT)
        # butterfly rows: store order (p, q, f) -> batch = q*128 + p
        op_ = res[c]["outp"][:, :FEAT]                  # two (p, q-half) stores
        H0 = op_[:NB // 2].reshape(P, M_B // 2, FEAT)
        H1 = op_[NB // 2:].reshape(P, M_B // 2, FEAT)
        obp = (np.concatenate([H0, H1], axis=1)         # [p, q, f]
                 .transpose(1, 0, 2).reshape(NB, FEAT))
        parts.append(np.concatenate([obm, obp], axis=0))
    full = np.concatenate(parts, axis=0).astype(np.float32)     # [B, 32]
    full = full.reshape(B_FULL, 4, 4, 2)                # [b, o, k, r]
    return tuple(np.ascontiguousarray(full[:, o]) for o in range(4))

